# revision 16
# baseline (speedup 1.0000x reference)
"""KeyValueMemory (top-k masked attention over memory slots) — Trainium2 Bass kernel.

Problem (hardcoded shapes):
  query  [8, 4096, 1024] f32      -> sharded over batch: 1 row per core, 8 cores
  keys   [512, 128] f32           -> replicated
  values [512, 1024] f32          -> replicated
  W_q    [128, 1024] f32, b_q [128] f32 -> replicated
  top_k = 8
Returns (retrieved [8,4096,1024] f32, weights [8,4096,512] f32).

Per-core pipeline (4096 tokens, 8 supertiles x 512 tokens):
  queryT (PE transpose, fp32) -> qproj (8 fp32 MMs, +b_q) -> scores = qT@keysT (fp32)
  -> top8 (DVE max) -> masked softmax (ACT exp + DVE select) -> weights out
  -> wT (PE transpose, rounded to f32r) -> retrieved = wT.T @ values (f32r MMs).
Scores path stays fp32 so the top-8 set matches the fp32 reference; the retrieved
matmul runs float32r (full PE rate, no discrete boundary to flip).
"""
import sys

if "/opt/trn_rl_repo" not in sys.path:
    sys.path.insert(0, "/opt/trn_rl_repo")

import numpy as np
from contextlib import ExitStack

import concourse.bacc as bacc
import concourse.tile as tile
from concourse import mybir
from concourse.bass_utils import run_bass_kernel_spmd
from concourse.masks import make_identity

F32 = mybir.dt.float32
F32R = mybir.dt.float32r
AF = mybir.ActivationFunctionType
ALU = mybir.AluOpType

N_CORES = 8
S_CORE = 4096          # tokens per core
V = 1024               # value/query dim
KD = 128               # key dim
NS = 512               # num slots
TOPK = 8
SUPER = 512            # tokens per supertile
N_SUPER = S_CORE // SUPER
C_SCALE = float(1.0 / np.sqrt(KD))

_BUILD_CACHE = {}


def _build():
    nc = bacc.Bacc("TRN2", target_bir_lowering=False, debug=False)

    query_d = nc.dram_tensor("query", [S_CORE, V], F32, kind="ExternalInput")
    keys_d = nc.dram_tensor("keys", [NS, KD], F32, kind="ExternalInput")
    values_d = nc.dram_tensor("values", [NS, V], F32, kind="ExternalInput")
    W_q_d = nc.dram_tensor("W_q", [KD, V], F32, kind="ExternalInput")
    b_q_d = nc.dram_tensor("b_q", [KD, 1], F32, kind="ExternalInput")
    retr_d = nc.dram_tensor("retrieved", [S_CORE, V], F32, kind="ExternalOutput")
    wout_d = nc.dram_tensor("weights", [S_CORE, NS], F32, kind="ExternalOutput")

    with tile.TileContext(nc) as tc:
        with ExitStack() as ctx:
            const = ctx.enter_context(tc.tile_pool(name="const", bufs=1))
            ld = ctx.enter_context(tc.tile_pool(name="ld", bufs=2))
            qin_p = ctx.enter_context(tc.tile_pool(name="qin", bufs=8))
            qT_p = ctx.enter_context(tc.tile_pool(name="qT", bufs=3))
            work = ctx.enter_context(tc.tile_pool(name="work", bufs=4))
            small = ctx.enter_context(tc.tile_pool(name="small", bufs=6))
            out_p = ctx.enter_context(tc.tile_pool(name="outp", bufs=4))
            ps_tr = ctx.enter_context(tc.tile_pool(name="ps_tr", bufs=2, space="PSUM"))
            ps_q = ctx.enter_context(tc.tile_pool(name="ps_q", bufs=1, space="PSUM"))
            ps_s = ctx.enter_context(tc.tile_pool(name="ps_s", bufs=2, space="PSUM"))
            ps_wt = ctx.enter_context(tc.tile_pool(name="ps_wt", bufs=1, space="PSUM"))
            ps_r = ctx.enter_context(tc.tile_pool(name="ps_r", bufs=2, space="PSUM"))

            # ---- constants / preprocessing (once) ----
            ident = const.tile([128, 128], F32)
            make_identity(nc, ident)

            b_q_sb = const.tile([KD, 1], F32)
            nc.sync.dma_start(out=b_q_sb, in_=b_q_d[:])

            # W_qT [128v_local, 8 chunks * 128k] fp32
            W_q_sb = const.tile([KD, V], F32)
            nc.sync.dma_start(out=W_q_sb, in_=W_q_d[:])
            W_qT = const.tile([128, 8 * KD], F32)
            for c in range(8):
                pt = ps_tr.tile([128, 128], F32, tag="qtr")
                nc.tensor.transpose(pt, W_q_sb[:, c * 128:(c + 1) * 128], ident)
                nc.scalar.copy(W_qT[:, c * KD:(c + 1) * KD], pt)

            # keysT [128k, 512s] fp32
            keysT = const.tile([KD, NS], F32)
            for j in range(4):
                kch = ld.tile([128, KD], F32, tag="kch")
                nc.sync.dma_start(out=kch, in_=keys_d[j * 128:(j + 1) * 128, :])
                pt = ps_tr.tile([128, 128], F32, tag="qtr")
                nc.tensor.transpose(pt, kch, ident)
                nc.scalar.copy(keysT[:, j * 128:(j + 1) * 128], pt)

            # values rounded to f32r: [128s_local, 4 chunks, 1024v]
            vals_r = const.tile([128, 4, V], F32R)
            for j in range(4):
                vch = ld.tile([128, V], F32, tag="vch")
                nc.sync.dma_start(out=vch, in_=values_d[j * 128:(j + 1) * 128, :])
                nc.scalar.copy(vals_r[:, j, :], vch)

            # ---- main loop ----
            for st in range(N_SUPER):
                t0 = st * SUPER

                qin = []
                for g in range(4):
                    qt = qin_p.tile([128, V], F32, tag="qin")
                    r0 = t0 + g * 128
                    nc.sync.dma_start(out=qt, in_=query_d[r0:r0 + 128, :])
                    qin.append(qt)

                # transpose query: queryT [128v_local, 8 chunks * 512t]
                queryT = qT_p.tile([128, 8 * SUPER], F32, tag="queryT")
                for c in range(8):
                    ptr = ps_tr.tile([128, SUPER], F32, tag="qtr")
                    for g in range(4):
                        nc.tensor.transpose(
                            ptr[:, g * 128:(g + 1) * 128],
                            qin[g][:, c * 128:(c + 1) * 128], ident)
                    if c % 2 == 0:
                        nc.scalar.copy(queryT[:, c * SUPER:(c + 1) * SUPER], ptr)
                    else:
                        nc.vector.tensor_copy(queryT[:, c * SUPER:(c + 1) * SUPER],
                                              ptr)

                # qproj: qT [128k, 512t] fp32, + b_q on eviction
                pq = ps_q.tile([KD, SUPER], F32, tag="pq")
                for c in range(8):
                    nc.tensor.matmul(
                        pq, W_qT[:, c * KD:(c + 1) * KD],
                        queryT[:, c * SUPER:(c + 1) * SUPER],
                        start=(c == 0), stop=(c == 7))
                qT_sb = qT_p.tile([KD, SUPER], F32, tag="qT_sb")
                nc.vector.tensor_scalar_add(qT_sb, pq, b_q_sb[:, 0:1])

                for g in range(4):
                    r0 = t0 + g * 128
                    # scores [128t, 512s] fp32 in PSUM
                    ps = ps_s.tile([128, NS], F32, tag="ps")
                    nc.tensor.matmul(ps, qT_sb[:, g * 128:(g + 1) * 128], keysT,
                                     start=True, stop=True)

                    # top-8 + masked softmax
                    top8 = small.tile([128, 8], F32, tag="top8")
                    nc.vector.max(out=top8, in_=ps)
                    neg_m = small.tile([128, 1], F32, tag="neg_m")
                    nc.vector.tensor_scalar_mul(neg_m, top8[:, 0:1], -C_SCALE)
                    e8 = small.tile([128, 8], F32, tag="e8")
                    denom = small.tile([128, 1], F32, tag="denom")
                    nc.scalar.activation(e8, top8, AF.Exp,
                                         bias=neg_m[:, 0:1], scale=C_SCALE,
                                         accum_out=denom[:, 0:1])
                    recip = small.tile([128, 1], F32, tag="recip")
                    nc.vector.reciprocal(recip, denom)
                    eP = work.tile([128, NS], F32, tag="eP")
                    nc.scalar.activation(eP, ps, AF.Exp,
                                         bias=neg_m[:, 0:1], scale=C_SCALE)
                    w_un = work.tile([128, NS], F32, tag="w_un")
                    nc.vector.scalar_tensor_tensor(
                        out=w_un, in0=ps, scalar=top8[:, 7:8], in1=eP,
                        op0=ALU.is_ge, op1=ALU.mult)
                    w = out_p.tile([128, NS], F32, tag="w")
                    nc.vector.tensor_scalar_mul(w, w_un, recip[:, 0:1])
                    nc.gpsimd.dma_start(out=wout_d[r0:r0 + 128, :], in_=w)

                    # wT [128s_local, 4 chunks * 128t], rounded to f32r on eviction
                    pwt = ps_wt.tile([128, NS], F32, tag="pwt")
                    for j in range(4):
                        nc.tensor.transpose(
                            pwt[:, j * 128:(j + 1) * 128],
                            w[:, j * 128:(j + 1) * 128], ident)
                    wT = work.tile([128, NS], F32R, tag="wT")
                    nc.scalar.copy(wT, pwt)

                    # retrieved [128t, 1024v] = sum_j wT_j.T @ values_j  (f32r)
                    retr = out_p.tile([128, V], F32, tag="retr")
                    for h in range(2):
                        pr = ps_r.tile([128, 512], F32, tag="pr")
                        for j in range(4):
                            nc.tensor.matmul(
                                pr, wT[:, j * 128:(j + 1) * 128],
                                vals_r[:, j, h * 512:(h + 1) * 512],
                                start=(j == 0), stop=(j == 3))
                        if h == 0:
                            nc.vector.tensor_copy(retr[:, 0:512], pr)
                        else:
                            nc.scalar.copy(retr[:, 512:1024], pr)
                    nc.gpsimd.dma_start(out=retr_d[r0:r0 + 128, :], in_=retr)

    nc.compile()
    return nc


def _get_nc():
    if "nc" not in _BUILD_CACHE:
        _BUILD_CACHE["nc"] = _build()
    return _BUILD_CACHE["nc"]


def _run(in_maps, trace=False, **kw):
    nc = _get_nc()
    return run_bass_kernel_spmd(nc, in_maps, core_ids=list(range(N_CORES)),
                                trace=trace, **kw)


def kernel(query, keys, values, W_q, b_q, top_k):
    assert int(top_k) == TOPK, f"kernel hardcodes top_k=8, got {top_k}"
    query = np.ascontiguousarray(np.asarray(query, dtype=np.float32))
    keys = np.ascontiguousarray(np.asarray(keys, dtype=np.float32))
    values = np.ascontiguousarray(np.asarray(values, dtype=np.float32))
    W_q = np.ascontiguousarray(np.asarray(W_q, dtype=np.float32))
    b_q = np.ascontiguousarray(np.asarray(b_q, dtype=np.float32)).reshape(KD, 1)
    B = query.shape[0]
    assert B == N_CORES and query.shape[1] == S_CORE and query.shape[2] == V

    in_maps = [{
        "query": query[i],
        "keys": keys,
        "values": values,
        "W_q": W_q,
        "b_q": b_q,
    } for i in range(N_CORES)]
    res = _run(in_maps)
    retrieved = np.stack([r["retrieved"] for r in res.results], axis=0)
    weights = np.stack([r["weights"] for r in res.results], axis=0)
    return retrieved, weights


# revision 18
# speedup vs baseline: 1.0062x; 1.0062x over previous
"""KeyValueMemory (top-k masked attention over memory slots) — Trainium2 Bass kernel.

Problem (hardcoded shapes):
  query  [8, 4096, 1024] f32      -> sharded over batch: 1 row per core, 8 cores
  keys   [512, 128] f32           -> replicated
  values [512, 1024] f32          -> replicated
  W_q    [128, 1024] f32, b_q [128] f32 -> replicated
  top_k = 8
Returns (retrieved [8,4096,1024] f32, weights [8,4096,512] f32).

Per-core pipeline (4096 tokens, 8 supertiles x 512 tokens):
  queryT (PE transpose, fp32) -> qproj (8 fp32 MMs, +b_q) -> scores = qT@keysT (fp32)
  -> top8 (DVE max) -> masked softmax (ACT exp + DVE select) -> weights out
  -> wT (PE transpose, rounded to f32r) -> retrieved = wT.T @ values (f32r MMs).
Scores path stays fp32 so the top-8 set matches the fp32 reference; the retrieved
matmul runs float32r (full PE rate, no discrete boundary to flip).
"""
import sys

if "/opt/trn_rl_repo" not in sys.path:
    sys.path.insert(0, "/opt/trn_rl_repo")

import numpy as np
from contextlib import ExitStack

import concourse.bacc as bacc
import concourse.tile as tile
from concourse import mybir
from concourse.bass_utils import run_bass_kernel_spmd
from concourse.masks import make_identity

F32 = mybir.dt.float32
F32R = mybir.dt.float32r
AF = mybir.ActivationFunctionType
ALU = mybir.AluOpType

N_CORES = 8
S_CORE = 4096          # tokens per core
V = 1024               # value/query dim
KD = 128               # key dim
NS = 512               # num slots
TOPK = 8
SUPER = 512            # tokens per supertile
N_SUPER = S_CORE // SUPER
C_SCALE = float(1.0 / np.sqrt(KD))

_BUILD_CACHE = {}


def _build():
    nc = bacc.Bacc("TRN2", target_bir_lowering=False, debug=False)

    query_d = nc.dram_tensor("query", [S_CORE, V], F32, kind="ExternalInput")
    keys_d = nc.dram_tensor("keys", [NS, KD], F32, kind="ExternalInput")
    values_d = nc.dram_tensor("values", [NS, V], F32, kind="ExternalInput")
    W_q_d = nc.dram_tensor("W_q", [KD, V], F32, kind="ExternalInput")
    b_q_d = nc.dram_tensor("b_q", [KD, 1], F32, kind="ExternalInput")
    retr_d = nc.dram_tensor("retrieved", [S_CORE, V], F32, kind="ExternalOutput")
    wout_d = nc.dram_tensor("weights", [S_CORE, NS], F32, kind="ExternalOutput")

    with tile.TileContext(nc) as tc:
        with ExitStack() as ctx:
            const = ctx.enter_context(tc.tile_pool(name="const", bufs=1))
            ld = ctx.enter_context(tc.tile_pool(name="ld", bufs=2))
            qin_p = ctx.enter_context(tc.tile_pool(name="qin", bufs=8))
            qT_p = ctx.enter_context(tc.tile_pool(name="qT", bufs=3))
            work = ctx.enter_context(tc.tile_pool(name="work", bufs=4))
            small = ctx.enter_context(tc.tile_pool(name="small", bufs=6))
            out_p = ctx.enter_context(tc.tile_pool(name="outp", bufs=4))
            ps_tr = ctx.enter_context(tc.tile_pool(name="ps_tr", bufs=2, space="PSUM"))
            ps_q = ctx.enter_context(tc.tile_pool(name="ps_q", bufs=1, space="PSUM"))
            ps_s = ctx.enter_context(tc.tile_pool(name="ps_s", bufs=2, space="PSUM"))
            ps_wt = ctx.enter_context(tc.tile_pool(name="ps_wt", bufs=1, space="PSUM"))
            ps_r = ctx.enter_context(tc.tile_pool(name="ps_r", bufs=2, space="PSUM"))

            # ---- constants / preprocessing (once) ----
            ident = const.tile([128, 128], F32)
            make_identity(nc, ident)

            b_q_sb = const.tile([KD, 1], F32)
            nc.sync.dma_start(out=b_q_sb, in_=b_q_d[:])

            # W_qT [128v_local, 8 chunks * 128k] fp32
            W_q_sb = const.tile([KD, V], F32)
            nc.sync.dma_start(out=W_q_sb, in_=W_q_d[:])
            W_qT = const.tile([128, 8 * KD], F32)
            for c in range(8):
                pt = ps_tr.tile([128, 128], F32, tag="qtr")
                nc.tensor.transpose(pt, W_q_sb[:, c * 128:(c + 1) * 128], ident)
                nc.scalar.copy(W_qT[:, c * KD:(c + 1) * KD], pt)

            # keysT [128k, 512s] fp32
            keysT = const.tile([KD, NS], F32)
            for j in range(4):
                kch = ld.tile([128, KD], F32, tag="kch")
                nc.sync.dma_start(out=kch, in_=keys_d[j * 128:(j + 1) * 128, :])
                pt = ps_tr.tile([128, 128], F32, tag="qtr")
                nc.tensor.transpose(pt, kch, ident)
                nc.scalar.copy(keysT[:, j * 128:(j + 1) * 128], pt)

            # values rounded to f32r: [128s_local, 4 chunks, 1024v]
            vals_r = const.tile([128, 4, V], F32R)
            for j in range(4):
                vch = ld.tile([128, V], F32, tag="vch")
                nc.sync.dma_start(out=vch, in_=values_d[j * 128:(j + 1) * 128, :])
                nc.scalar.copy(vals_r[:, j, :], vch)

            # ---- main loop ----
            for st in range(N_SUPER):
                t0 = st * SUPER

                qin = []
                for g in range(4):
                    qt = qin_p.tile([128, V], F32, tag="qin")
                    r0 = t0 + g * 128
                    nc.sync.dma_start(out=qt, in_=query_d[r0:r0 + 128, :])
                    qin.append(qt)

                # transpose query: queryT [128v_local, 8 chunks * 512t]
                queryT = qT_p.tile([128, 8 * SUPER], F32, tag="queryT")
                for c in range(8):
                    ptr = ps_tr.tile([128, SUPER], F32, tag="qtr")
                    for g in range(4):
                        nc.tensor.transpose(
                            ptr[:, g * 128:(g + 1) * 128],
                            qin[g][:, c * 128:(c + 1) * 128], ident)
                    if c % 2 == 0:
                        nc.scalar.copy(queryT[:, c * SUPER:(c + 1) * SUPER], ptr)
                    else:
                        nc.vector.tensor_copy(queryT[:, c * SUPER:(c + 1) * SUPER],
                                              ptr)

                # qproj: qT [128k, 512t] fp32, + b_q on eviction
                pq = ps_q.tile([KD, SUPER], F32, tag="pq")
                for c in range(8):
                    nc.tensor.matmul(
                        pq, W_qT[:, c * KD:(c + 1) * KD],
                        queryT[:, c * SUPER:(c + 1) * SUPER],
                        start=(c == 0), stop=(c == 7))
                qT_sb = qT_p.tile([KD, SUPER], F32, tag="qT_sb")
                nc.vector.tensor_scalar_add(qT_sb, pq, b_q_sb[:, 0:1])

                for g in range(4):
                    r0 = t0 + g * 128
                    # scores [128t, 512s] fp32 in PSUM
                    ps = ps_s.tile([128, NS], F32, tag="ps")
                    nc.tensor.matmul(ps, qT_sb[:, g * 128:(g + 1) * 128], keysT,
                                     start=True, stop=True)

                    # top-8 + masked softmax
                    top8 = small.tile([128, 8], F32, tag="top8")
                    nc.vector.max(out=top8, in_=ps)
                    neg_m = small.tile([128, 1], F32, tag="neg_m")
                    nc.vector.tensor_scalar_mul(neg_m, top8[:, 0:1], -C_SCALE)
                    e8 = small.tile([128, 8], F32, tag="e8")
                    denom = small.tile([128, 1], F32, tag="denom")
                    nc.scalar.activation(e8, top8, AF.Exp,
                                         bias=neg_m[:, 0:1], scale=C_SCALE,
                                         accum_out=denom[:, 0:1])
                    recip = small.tile([128, 1], F32, tag="recip")
                    nc.vector.reciprocal(recip, denom)
                    eP = work.tile([128, NS], F32, tag="eP")
                    nc.scalar.activation(eP, ps, AF.Exp,
                                         bias=neg_m[:, 0:1], scale=C_SCALE)
                    w_un = work.tile([128, NS], F32, tag="w_un")
                    nc.vector.scalar_tensor_tensor(
                        out=w_un, in0=ps, scalar=top8[:, 7:8], in1=eP,
                        op0=ALU.is_ge, op1=ALU.mult)
                    w = out_p.tile([128, NS], F32, tag="w")
                    nc.vector.tensor_scalar_mul(w, w_un, recip[:, 0:1])
                    nc.gpsimd.dma_start(out=wout_d[r0:r0 + 128, :], in_=w)

                    # wT [128s_local, 4 chunks * 128t], rounded to f32r on eviction
                    pwt = ps_wt.tile([128, NS], F32, tag="pwt")
                    for j in range(4):
                        nc.tensor.transpose(
                            pwt[:, j * 128:(j + 1) * 128],
                            w[:, j * 128:(j + 1) * 128], ident)
                    wT = work.tile([128, NS], F32R, tag="wT")
                    nc.scalar.copy(wT, pwt)

                    # retrieved [128t, 1024v] = sum_j wT_j.T @ values_j  (f32r)
                    retr = out_p.tile([128, V], F32, tag="retr")
                    for h in range(2):
                        pr = ps_r.tile([128, 512], F32, tag="pr")
                        for j in range(4):
                            nc.tensor.matmul(
                                pr, wT[:, j * 128:(j + 1) * 128],
                                vals_r[:, j, h * 512:(h + 1) * 512],
                                start=(j == 0), stop=(j == 3))
                        if h == 0:
                            nc.vector.tensor_copy(retr[:, 0:512], pr)
                        else:
                            nc.scalar.copy(retr[:, 512:1024], pr)
                    nc.gpsimd.dma_start(out=retr_d[r0:r0 + 128, :], in_=retr)

    nc.compile()
    return nc


def _get_nc():
    if "nc" not in _BUILD_CACHE:
        _BUILD_CACHE["nc"] = _build()
    return _BUILD_CACHE["nc"]


def _run(in_maps, trace=False, **kw):
    nc = _get_nc()
    return run_bass_kernel_spmd(nc, in_maps, core_ids=list(range(N_CORES)),
                                trace=trace, **kw)


def kernel(query, keys, values, W_q, b_q, top_k):
    assert int(top_k) == TOPK, f"kernel hardcodes top_k=8, got {top_k}"
    query = np.ascontiguousarray(np.asarray(query, dtype=np.float32))
    keys = np.ascontiguousarray(np.asarray(keys, dtype=np.float32))
    values = np.ascontiguousarray(np.asarray(values, dtype=np.float32))
    W_q = np.ascontiguousarray(np.asarray(W_q, dtype=np.float32))
    b_q = np.ascontiguousarray(np.asarray(b_q, dtype=np.float32)).reshape(KD, 1)
    B = query.shape[0]
    assert B == N_CORES and query.shape[1] == S_CORE and query.shape[2] == V

    in_maps = [{
        "query": query[i],
        "keys": keys,
        "values": values,
        "W_q": W_q,
        "b_q": b_q,
    } for i in range(N_CORES)]
    res = _run(in_maps)
    retrieved = np.stack([r["retrieved"] for r in res.results], axis=0)
    weights = np.stack([r["weights"] for r in res.results], axis=0)
    return retrieved, weights
